# revision 17
# baseline (speedup 1.0000x reference)
"""Trainium2 Bass kernel for nn_DotProductAttention_2465311228070 (sparse attention).

Math (per batch b):
  scores = 0.25 * (Q Wq_low + bq)(K Wk_low + bk)^T            [s, s]
  masked  = scores + NEG * onehot(vl[k] == q)
  top8(q) = 8 largest keys of row q of masked
  f[k]    = 0.25 * <Q[k] Wq_high + bqh, K[k] Wk_high + bkh>   (depends on k only!)
  corrected[q, k] = f[k] if k in top8(q) else masked[q, k]
  attn    = softmax(corrected, axis=q)     (normalize down columns)
  out     = attn @ V

Sharding: 8 cores = (batch b = c//2) x (query half h = c%2). Each core computes
its [1024 q, 2048 k] block fully; only the softmax column-sum needs a pairwise
(2-core) AllReduce of 8KB.

Self-contained: hardcodes shapes; only imports the system concourse install.
"""

import sys
from contextlib import ExitStack

import numpy as np

if "/opt/trn_rl_repo" not in sys.path:
    sys.path.insert(0, "/opt/trn_rl_repo")

B, S, D, DL = 4, 2048, 128, 16
QH = S // 2          # rows per core
QC = QH // 128       # 8 query chunks per core
KC = S // 128        # 16 key chunks
N_CORES = 8
NEG = -1000.0        # exp(score + NEG) == 0 in fp32 for any |score| < 900

_cache: dict = {}


def _build(single_core: bool = False):
    import concourse.bass as bass
    import concourse.tile as tile
    from concourse import bacc, mybir
    from concourse.masks import make_identity

    f32 = mybir.dt.float32
    f16 = mybir.dt.float16
    i32 = mybir.dt.int32
    AF = mybir.ActivationFunctionType
    OP = mybir.AluOpType
    f32r = mybir.dt.float32r
    R = lambda ap: ap  # tiles feeding matmuls are declared float32r below

    nc = bacc.Bacc("TRN2", debug=False, num_devices=1 if single_core else N_CORES)

    # ---- kernel I/O ----
    t_q = nc.dram_tensor("queries", [S, D], f32, kind="ExternalInput")
    t_ql = nc.dram_tensor("queries_loc", [QH, D], f32, kind="ExternalInput")
    t_k = nc.dram_tensor("keys", [S, D], f32, kind="ExternalInput")
    t_v = nc.dram_tensor("values", [S, D], f32, kind="ExternalInput")
    t_vl = nc.dram_tensor("valid_lens", [1, S], i32, kind="ExternalInput")
    t_qb = nc.dram_tensor("qbase", [1, 1], f32, kind="ExternalInput")
    t_wql = nc.dram_tensor("Wq_low", [D, DL], f32, kind="ExternalInput")
    t_bql = nc.dram_tensor("bq_low", [DL, 1], f32, kind="ExternalInput")
    t_wkl = nc.dram_tensor("Wk_low", [D, DL], f32, kind="ExternalInput")
    t_bkl = nc.dram_tensor("bk_low", [DL, 1], f32, kind="ExternalInput")
    t_wqh = nc.dram_tensor("Wq_high", [D, DL], f32, kind="ExternalInput")
    t_bqh = nc.dram_tensor("bq_high", [DL, 1], f32, kind="ExternalInput")
    t_wkh = nc.dram_tensor("Wk_high", [D, DL], f32, kind="ExternalInput")
    t_bkh = nc.dram_tensor("bk_high", [DL, 1], f32, kind="ExternalInput")
    t_out = nc.dram_tensor("out", [QH, D], f32, kind="ExternalOutput")

    # collective bounce buffers (internal DRAM)
    cs_in = nc.dram_tensor("cs_in", [128, KC], f32)
    cs_out = nc.dram_tensor("cs_out", [128, KC], f32)
    RG = [[0, 1], [2, 3], [4, 5], [6, 7]]

    with tile.TileContext(nc) as tc, ExitStack() as ctx:
        const = ctx.enter_context(tc.tile_pool(name="const", bufs=1))
        ps_tr = ctx.enter_context(tc.tile_pool(name="ps_tr", bufs=3, space="PSUM"))

        # ---- constants ----
        ident = const.tile([128, 128], f32, tag="ident")
        make_identity(nc, ident[:])
        negI = const.tile([128, 128], f16, tag="negI")
        nc.gpsimd.memset(negI[:], 0.0)
        nc.gpsimd.affine_select(
            out=negI[:], in_=negI[:], pattern=[[-1, 128]],
            compare_op=OP.not_equal, fill=NEG, base=0, channel_multiplier=1,
        )
        ones16 = const.tile([DL, 1], f32, tag="ones16")
        nc.vector.memset(ones16[:], 1.0)

        # ---- small loads ----
        w_ql = const.tile([D, DL], f32, tag="w_ql")
        w_kl = const.tile([D, DL], f32, tag="w_kl")
        w_qh = const.tile([D, DL], f32, tag="w_qh")
        w_kh = const.tile([D, DL], f32, tag="w_kh")
        b_ql = const.tile([DL, 1], f32, tag="b_ql")
        b_kl = const.tile([DL, 1], f32, tag="b_kl")
        b_qh = const.tile([DL, 1], f32, tag="b_qh")
        b_kh = const.tile([DL, 1], f32, tag="b_kh")
        for t_w, w in ((t_wql, w_ql), (t_wkl, w_kl), (t_wqh, w_qh), (t_wkh, w_kh)):
            nc.sync.dma_start(w[:], t_w[:, :])
        for t_b, bb in ((t_bql, b_ql), (t_bkl, b_kl), (t_bqh, b_qh), (t_bkh, b_kh)):
            nc.sync.dma_start(bb[:], t_b[:, :])
        # fold the 1/sqrt(DL)=0.25 score scale into the q-side low projection
        w_qls = const.tile([D, DL], f32, tag="w_qls")
        nc.vector.tensor_scalar(out=w_qls[:], in0=w_ql[:], scalar1=0.25,
                                scalar2=None, op0=OP.mult)
        b_qls = const.tile([DL, 1], f32, tag="b_qls")
        nc.vector.tensor_scalar(out=b_qls[:], in0=b_ql[:], scalar1=0.25,
                                scalar2=None, op0=OP.mult)

        # valid_lens -> clipped f16 row, broadcast across partitions
        vl_bc = const.tile([128, S], f16, tag="vl_bc")
        with tc.tile_pool(name="vltmp", bufs=1) as vltmp:
            vl_i = vltmp.tile([1, S], i32, tag="vl_i")
            nc.sync.dma_start(vl_i[:], t_vl[:, :])
            vl_f32 = vltmp.tile([1, S], f32, tag="vl_f32")
            nc.vector.tensor_copy(vl_f32[:], vl_i[:])
            vl_f16 = vltmp.tile([1, S], f16, tag="vl_f16")
            nc.vector.tensor_scalar(out=vl_f16[:], in0=vl_f32[:], scalar1=float(S - 1),
                                    scalar2=None, op0=OP.min)
            nc.gpsimd.partition_broadcast(vl_bc[:], vl_f16[0:1, :])

        # global query index of each (partition, q-chunk): qbase + p + 128*qc
        qb_t = const.tile([1, 1], f32, tag="qb_t")
        nc.sync.dma_start(qb_t[:], t_qb[:, :])
        qb_bc = const.tile([128, 1], f32, tag="qb_bc")
        nc.gpsimd.partition_broadcast(qb_bc[:], qb_t[0:1, :])
        qi_i = const.tile([128, QC], i32, tag="qi_i")
        nc.gpsimd.iota(qi_i[:], pattern=[[128, QC]], base=0, channel_multiplier=1)
        qi_f32 = const.tile([128, QC], f32, tag="qi_f32")
        nc.vector.tensor_copy(qi_f32[:], qi_i[:])
        qidx = const.tile([128, QC], f32, tag="qidx")
        nc.vector.tensor_scalar(out=qidx[:], in0=qi_f32[:], scalar1=qb_bc[:, 0:1],
                                scalar2=None, op0=OP.add)

        # ---- load + transpose Q (full), K (full), Q_loc; V loads straight ----
        qT = const.tile([128, S], f32, tag="qT")     # [d, q]
        kT = const.tile([128, S], f32, tag="kT")     # [d, k]
        qTl = const.tile([128, QH], f32, tag="qTl")  # [d, q_local]
        v_all = const.tile([128, KC * 128], f32, tag="v_all")  # [k_in_chunk, kc*128+d]

        with tc.tile_pool(name="inp", bufs=1) as inp:
            # one big rearranged DMA per tensor, spread across the SP / ACT /
            # POOL DGE rings so the loads run in parallel
            q_nat = inp.tile([128, KC * 128], f32, tag="q_nat")
            k_nat = inp.tile([128, KC * 128], f32, tag="k_nat")
            ql_nat = inp.tile([128, QC * 128], f32, tag="ql_nat")
            nc.sync.dma_start(q_nat[:].rearrange("p (c d) -> p c d", d=128),
                              t_q[:, :].rearrange("(c p) d -> p c d", p=128))
            nc.scalar.dma_start(k_nat[:].rearrange("p (c d) -> p c d", d=128),
                                t_k[:, :].rearrange("(c p) d -> p c d", p=128))
            nc.gpsimd.dma_start(ql_nat[:].rearrange("p (c d) -> p c d", d=128),
                                t_ql[:, :].rearrange("(c p) d -> p c d", p=128))
            nc.gpsimd.dma_start(v_all[:].rearrange("p (c d) -> p c d", d=128),
                                t_v[:, :].rearrange("(c p) d -> p c d", p=128))
            for dst, stg, nch in ((qT, q_nat, KC), (kT, k_nat, KC), (qTl, ql_nat, QC)):
                for g in range(nch // 4):
                    ps = ps_tr.tile([128, 512], f32, tag="tr")
                    for j in range(4):
                        c = g * 4 + j
                        nc.tensor.transpose(ps[:, j * 128:(j + 1) * 128],
                                            stg[:, c * 128:(c + 1) * 128], ident[:])
                    nc.scalar.activation(dst[:, g * 512:(g + 1) * 512], ps[:], AF.Copy)

        # ---- projections ----
        qlowT = const.tile([DL, QH], f32, tag="qlowT")   # 0.25*(Wq_low^T Q_loc^T + bq)
        klowT = const.tile([DL, S], f32, tag="klowT")
        F_bc = const.tile([128, S], f32, tag="F_bc")

        with tc.tile_pool(name="proj", bufs=1) as proj, \
             tc.tile_pool(name="ps_p", bufs=2, space="PSUM") as ps_p, \
             tc.tile_pool(name="ps_f", bufs=2, space="PSUM") as ps_f:
            qpT = proj.tile([DL, S], f32, tag="qpT")
            F_row = proj.tile([1, S], f32, tag="F_row")     # exp(f[k])
            kpT = proj.tile([DL, S], f32, tag="kpT")
            prod = proj.tile([DL, S], f32, tag="prod")
            for s in range(2):
                ps = ps_p.tile([DL, 512], f32, tag="pp")
                nc.tensor.matmul(ps[:], R(w_qls[:]), R(qTl[:, s * 512:(s + 1) * 512]))
                nc.scalar.activation(qlowT[:, s * 512:(s + 1) * 512], ps[:], AF.Identity,
                                     bias=b_qls[:, 0:1])
            for s in range(4):
                sl = slice(s * 512, (s + 1) * 512)
                ps = ps_p.tile([DL, 512], f32, tag="pp")
                nc.tensor.matmul(ps[:], R(w_kl[:]), R(kT[:, sl]))
                nc.scalar.activation(klowT[:, sl], ps[:], AF.Identity, bias=b_kl[:, 0:1])
                ps2 = ps_p.tile([DL, 512], f32, tag="pp")
                nc.tensor.matmul(ps2[:], R(w_qh[:]), R(qT[:, sl]))
                nc.scalar.activation(qpT[:, sl], ps2[:], AF.Identity, bias=b_qh[:, 0:1])
                ps3 = ps_p.tile([DL, 512], f32, tag="pp")
                nc.tensor.matmul(ps3[:], R(w_kh[:]), R(kT[:, sl]))
                nc.scalar.activation(kpT[:, sl], ps3[:], AF.Identity, bias=b_kh[:, 0:1])
            nc.vector.tensor_tensor(out=prod[:], in0=qpT[:], in1=kpT[:], op=OP.mult)
            for s in range(4):
                sl = slice(s * 512, (s + 1) * 512)
                psf = ps_f.tile([1, 512], f32, tag="pf")
                nc.tensor.matmul(psf[:], R(ones16[:]), R(prod[:, sl]))
                nc.scalar.activation(F_row[0:1, sl], psf[:], AF.Exp, scale=0.25)
            nc.gpsimd.partition_broadcast(F_bc[:], F_row[0:1, :])

        # ---- phase A: scores -> exp -> top8 -> correct ;  phase B: transpose ----
        E_T = const.tile([128, KC * QH], f32r, tag="E_T")  # [k_in_chunk, kc*1024+q]
        cs = const.tile([128, 2 * KC], f32, tag="cs")     # per-(kc, wave) colsums

        ec = ctx.enter_context(tc.tile_pool(name="ec", bufs=5))
        ohp = ctx.enter_context(tc.tile_pool(name="oh", bufs=2))
        m8p = ctx.enter_context(tc.tile_pool(name="m8", bufs=2))
        t8p = ctx.enter_context(tc.tile_pool(name="t8", bufs=4))
        E_tiles = [None] * QC

        with tc.tile_pool(name="ps_sc", bufs=2, space="PSUM") as ps_sc:
            for qc in range(QC):
                oh = ohp.tile([128, S], f16, tag="oh")
                nc.vector.tensor_scalar(out=oh[:], in0=vl_bc[:],
                                        scalar1=qidx[:, qc:qc + 1],
                                        scalar2=None, op0=OP.is_equal)
                E = ec.tile([128, S], f32, tag="E")
                E_tiles[qc] = E
                for h2 in range(2):
                    ps = ps_sc.tile([128, 1024], f32, tag="sc")
                    for s in range(2):
                        col = h2 * 1024 + s * 512
                        nc.tensor.matmul(ps[:, s * 512:(s + 1) * 512],
                                         R(qlowT[:, qc * 128:(qc + 1) * 128]),
                                         R(klowT[:, col:col + 512]),
                                         start=True, stop=False)
                        nc.tensor.matmul(ps[:, s * 512:(s + 1) * 512],
                                         negI[:], oh[:, col:col + 512],
                                         start=False, stop=True)
                    nc.scalar.activation(E[:, h2 * 1024:(h2 + 1) * 1024], ps[:], AF.Exp)
                t8 = t8p.tile([128, 8], f32, tag="t8")
                nc.vector.max(out=t8[:], in_=E[:])
                m8 = m8p.tile([128, S], mybir.dt.uint8, tag="m8")
                nc.gpsimd.tensor_scalar(out=m8[:], in0=E[:], scalar1=t8[:, 7:8],
                                        scalar2=None, op0=OP.is_ge)
                nc.vector.copy_predicated(out=E[:], mask=m8[:], data=F_bc[:])

                if qc == 3 or qc == 7:
                    w = qc // 4
                    for kc in range(KC):
                        ps = ps_tr.tile([128, 512], f32, tag="tr")
                        for j in range(4):
                            Ej = E_tiles[w * 4 + j]
                            nc.tensor.transpose(ps[:, j * 128:(j + 1) * 128],
                                                Ej[:, kc * 128:(kc + 1) * 128],
                                                ident[:])
                        idx = kc * 2 + w
                        nc.scalar.activation(
                            E_T[:, kc * QH + w * 512: kc * QH + w * 512 + 512],
                            ps[:], AF.Copy, accum_out=cs[:, idx:idx + 1])

        # ---- phase C: column sums -> pairwise AllReduce -> 1/colsum -> scale V ----
        colsum = const.tile([128, KC], f32, tag="colsum")
        nc.vector.tensor_reduce(out=colsum[:], in_=cs[:].rearrange("p (k w) -> p k w", w=2),
                                axis=mybir.AxisListType.X, op=OP.add)
        nc.sync.dma_start(cs_in[:, :], colsum[:])
        if single_core:
            # profiling-only variant: stand in for the pairwise AllReduce
            nc.sync.dma_start(cs_out[:, :], cs_in[:, :])
        else:
            nc.gpsimd.collective_compute(
                "AllReduce", OP.add, replica_groups=RG,
                ins=[cs_in.ap()], outs=[cs_out.ap()],
            )
        ctot = const.tile([128, KC], f32, tag="ctot")
        nc.sync.dma_start(ctot[:], cs_out[:, :])
        rcol = const.tile([128, KC], f32, tag="rcol")
        nc.vector.reciprocal(rcol[:], ctot[:])
        v_sc = const.tile([128, KC * 128], f32r, tag="v_sc")
        for kc in range(KC):
            nc.vector.tensor_scalar(out=v_sc[:, kc * 128:(kc + 1) * 128],
                                   in0=v_all[:, kc * 128:(kc + 1) * 128],
                                   scalar1=rcol[:, kc:kc + 1],
                                   scalar2=None, op0=OP.mult)

        # ---- phase D: out^T = sum_k v_sc[k,:]^T E_T[k,:] ; transpose back; store ----
        outT = const.tile([128, QH], f32, tag="outT")   # [d, q_local]
        out_sb = const.tile([128, QH], f32, tag="out_sb")  # [q_in_chunk, qc*128+d]
        with tc.tile_pool(name="ps_o", bufs=2, space="PSUM") as ps_o:
            for h2 in range(2):
                po = ps_o.tile([128, 512], f32, tag="po")
                for kc in range(KC):
                    nc.tensor.matmul(po[:], R(v_sc[:, kc * 128:(kc + 1) * 128]),
                                     R(E_T[:, kc * QH + h2 * 512: kc * QH + h2 * 512 + 512]),
                                     start=(kc == 0), stop=(kc == KC - 1))
                nc.scalar.activation(outT[:, h2 * 512:(h2 + 1) * 512], po[:], AF.Copy)
            for g in range(2):
                ps = ps_tr.tile([128, 512], f32, tag="tr")
                for j in range(4):
                    qc = g * 4 + j
                    nc.tensor.transpose(ps[:, j * 128:(j + 1) * 128],
                                        outT[:, qc * 128:(qc + 1) * 128], ident[:])
                nc.scalar.activation(out_sb[:, g * 512:(g + 1) * 512], ps[:], AF.Copy)
        # one rearranged store per half, on separate DGE rings
        nc.sync.dma_start(
            t_out[0:QH // 2, :].rearrange("(c p) d -> p c d", p=128),
            out_sb[:, 0:QH // 2].rearrange("p (c d) -> p c d", d=128))
        nc.scalar.dma_start(
            t_out[QH // 2:QH, :].rearrange("(c p) d -> p c d", p=128),
            out_sb[:, QH // 2:QH].rearrange("p (c d) -> p c d", d=128))

    nc.compile()
    return nc


def _make_in_maps(inputs):
    q = np.ascontiguousarray(np.asarray(inputs["queries"], dtype=np.float32))
    k = np.ascontiguousarray(np.asarray(inputs["keys"], dtype=np.float32))
    v = np.ascontiguousarray(np.asarray(inputs["values"], dtype=np.float32))
    vl = np.ascontiguousarray(np.asarray(inputs["valid_lens"], dtype=np.int32))
    ws = {n: np.ascontiguousarray(np.asarray(inputs[n], dtype=np.float32))
          for n in ("Wq_low", "Wk_low", "Wq_high", "Wk_high")}
    bs = {n: np.ascontiguousarray(
            np.asarray(inputs[n], dtype=np.float32).reshape(DL, 1))
          for n in ("bq_low", "bk_low", "bq_high", "bk_high")}
    in_maps = []
    for c in range(N_CORES):
        b, h = c // 2, c % 2
        m = {
            "queries": q[b],
            "queries_loc": np.ascontiguousarray(q[b, h * QH:(h + 1) * QH]),
            "keys": k[b],
            "values": v[b],
            "valid_lens": vl[b].reshape(1, S),
            "qbase": np.array([[float(h * QH)]], np.float32),
        }
        m.update(ws)
        m.update(bs)
        in_maps.append(m)
    return in_maps


def kernel(**inputs) -> np.ndarray:
    from concourse import bass_utils

    if "nc" not in _cache:
        _cache["nc"] = _build()
    nc = _cache["nc"]
    in_maps = _make_in_maps(inputs)
    res = bass_utils.run_bass_kernel_spmd(nc, in_maps, core_ids=list(range(N_CORES)))
    out = np.empty((B, S, D), np.float32)
    for c in range(N_CORES):
        b, h = c // 2, c % 2
        out[b, h * QH:(h + 1) * QH, :] = res.results[c]["out"]
    return out


# revision 34
# speedup vs baseline: 163.1867x; 163.1867x over previous
"""Trainium2 Bass kernel for nn_DotProductAttention_2465311228070 (sparse attention).

Math (per batch b):
  scores = 0.25 * (Q Wq_low + bq)(K Wk_low + bk)^T            [s, s]
  masked  = scores + NEG * onehot(vl[k] == q)
  top8(q) = 8 largest keys of row q of masked
  f[k]    = 0.25 * <Q[k] Wq_high + bqh, K[k] Wk_high + bkh>   (depends on k only!)
  corrected[q, k] = f[k] if k in top8(q) else masked[q, k]
  attn    = softmax(corrected, axis=q)     (normalize down columns)
  out     = attn @ V

Sharding: 8 cores = (batch b = c//2) x (query half h = c%2). Each core computes
its [1024 q, 2048 k] block fully; only the softmax column-sum needs a pairwise
(2-core) AllReduce of 8KB.

Self-contained: hardcodes shapes; only imports the system concourse install.
"""

import os
import sys
from contextlib import ExitStack

import numpy as np

if "/opt/trn_rl_repo" not in sys.path:
    sys.path.insert(0, "/opt/trn_rl_repo")

B, S, D, DL = 4, 2048, 128, 16
QH = S // 2          # rows per core
QC = QH // 128       # 8 query chunks per core
KC = S // 128        # 16 key chunks
N_CORES = 8
NEG = -1000.0        # exp(score + NEG) == 0 in fp32 for any |score| < 900

_cache: dict = {}


def _build(single_core: bool = False):
    import concourse.bass as bass
    import concourse.tile as tile
    from concourse import bacc, mybir
    from concourse.masks import make_identity

    f32 = mybir.dt.float32
    f16 = mybir.dt.float16
    i32 = mybir.dt.int32
    AF = mybir.ActivationFunctionType
    OP = mybir.AluOpType
    f32r = mybir.dt.float32r
    R = lambda ap: ap  # tiles feeding matmuls are declared float32r below

    nc = bacc.Bacc("TRN2", debug=False, num_devices=1 if single_core else N_CORES)

    # ---- kernel I/O ----
    t_q = nc.dram_tensor("queries", [S, D], f32, kind="ExternalInput")
    t_ql = nc.dram_tensor("queries_loc", [QH, D], f32, kind="ExternalInput")
    t_k = nc.dram_tensor("keys", [S, D], f32, kind="ExternalInput")
    t_v = nc.dram_tensor("values", [S, D], f32, kind="ExternalInput")
    t_vl = nc.dram_tensor("valid_lens", [1, S], i32, kind="ExternalInput")
    t_qb = nc.dram_tensor("qbase", [1, 1], f32, kind="ExternalInput")
    t_wql = nc.dram_tensor("Wq_low", [D, DL], f32, kind="ExternalInput")
    t_bql = nc.dram_tensor("bq_low", [DL, 1], f32, kind="ExternalInput")
    t_wkl = nc.dram_tensor("Wk_low", [D, DL], f32, kind="ExternalInput")
    t_bkl = nc.dram_tensor("bk_low", [DL, 1], f32, kind="ExternalInput")
    t_wqh = nc.dram_tensor("Wq_high", [D, DL], f32, kind="ExternalInput")
    t_bqh = nc.dram_tensor("bq_high", [DL, 1], f32, kind="ExternalInput")
    t_wkh = nc.dram_tensor("Wk_high", [D, DL], f32, kind="ExternalInput")
    t_bkh = nc.dram_tensor("bk_high", [DL, 1], f32, kind="ExternalInput")
    t_out = nc.dram_tensor("out", [QH, D], f32, kind="ExternalOutput")

    # collective bounce buffers (internal DRAM)
    cs_in0 = nc.dram_tensor("cs_in0", [128, KC // 2], f32)
    cs_out0 = nc.dram_tensor("cs_out0", [128, KC // 2], f32)
    cs_in1 = nc.dram_tensor("cs_in1", [128, KC // 2], f32)
    cs_out1 = nc.dram_tensor("cs_out1", [128, KC // 2], f32)
    RG = [[0, 1], [2, 3], [4, 5], [6, 7]]

    with tile.TileContext(nc) as tc, ExitStack() as ctx:
        const = ctx.enter_context(tc.tile_pool(name="const", bufs=1))
        ps_tr = ctx.enter_context(tc.tile_pool(name="ps_tr", bufs=4, space="PSUM"))

        # ---- constants ----
        ident = const.tile([128, 128], f32, tag="ident")
        make_identity(nc, ident[:])
        negI = const.tile([128, 128], f16, tag="negI")
        nc.gpsimd.memset(negI[:], 0.0)
        nc.gpsimd.affine_select(
            out=negI[:], in_=negI[:], pattern=[[-1, 128]],
            compare_op=OP.not_equal, fill=NEG, base=0, channel_multiplier=1,
        )
        ones16 = const.tile([DL, 1], f32, tag="ones16")
        nc.vector.memset(ones16[:], 1.0)

        # ---- tiles for small inputs ----
        w_ql = const.tile([D, DL], f32, tag="w_ql")
        w_kl = const.tile([D, DL], f32, tag="w_kl")
        w_qh = const.tile([D, DL], f32r, tag="w_qh")
        w_kh = const.tile([D, DL], f32r, tag="w_kh")
        w_qh_s = const.tile([D, DL], f32, tag="w_qh_s")
        w_kh_s = const.tile([D, DL], f32, tag="w_kh_s")
        b_ql = const.tile([DL, 1], f32, tag="b_ql")
        b_kl = const.tile([DL, 1], f32, tag="b_kl")
        b_qh = const.tile([DL, 1], f32, tag="b_qh")
        b_kh = const.tile([DL, 1], f32, tag="b_kh")
        vl_bc = const.tile([128, S], f16, tag="vl_bc")
        qb_t = const.tile([1, 1], f32, tag="qb_t")
        qidx = const.tile([128, QC], f32, tag="qidx")
        qT = const.tile([128, S], f32, tag="qT")     # [d, q]
        kT = const.tile([128, S], f32, tag="kT")     # [d, k]
        qTl = const.tile([128, QH], f32, tag="qTl")  # [d, q_local]
        v_all = const.tile([128, KC * 128], f32, tag="v_all")  # [k_in_chunk, kc*128+d]

        with tc.tile_pool(name="inp", bufs=1) as inp:
            # one big rearranged DMA per tensor, spread across the SP and ACT
            # DGE rings so the loads run in parallel; smalls queue behind them
            q_nat = inp.tile([128, KC * 128], f32, tag="q_nat")
            k_nat = inp.tile([128, KC * 128], f32, tag="k_nat")
            ql_nat = inp.tile([128, QC * 128], f32, tag="ql_nat")
            vl_i = inp.tile([1, S], i32, tag="vl_i")
            nc.sync.dma_start(q_nat[:].rearrange("p (c d) -> p c d", d=128),
                              t_q[:, :].rearrange("(c p) d -> p c d", p=128))
            nc.scalar.dma_start(ql_nat[:].rearrange("p (c d) -> p c d", d=128),
                                t_ql[:, :].rearrange("(c p) d -> p c d", p=128))
            nc.scalar.dma_start(k_nat[:].rearrange("p (c d) -> p c d", d=128),
                                t_k[:, :].rearrange("(c p) d -> p c d", p=128))
            nc.sync.dma_start(v_all[:].rearrange("p (c d) -> p c d", d=128),
                              t_v[:, :].rearrange("(c p) d -> p c d", p=128))
            nc.sync.dma_start(vl_i[:], t_vl[:, :])
            nc.sync.dma_start(qb_t[:], t_qb[:, :])
            for t_w, w in ((t_wql, w_ql), (t_wkl, w_kl)):
                nc.sync.dma_start(w[:], t_w[:, :])
            for t_w, w in ((t_wqh, w_qh_s), (t_wkh, w_kh_s)):
                nc.scalar.dma_start(w[:], t_w[:, :])
            for t_b, bb in ((t_bql, b_ql), (t_bkl, b_kl)):
                nc.sync.dma_start(bb[:], t_b[:, :])
            for t_b, bb in ((t_bqh, b_qh), (t_bkh, b_kh)):
                nc.scalar.dma_start(bb[:], t_b[:, :])

            # weight prep
            nc.vector.tensor_copy(w_qh[:], w_qh_s[:])
            nc.vector.tensor_copy(w_kh[:], w_kh_s[:])
            # fold the 1/sqrt(DL)=0.25 score scale into the q-side low projection
            w_qls = const.tile([D, DL], f32, tag="w_qls")
            nc.vector.tensor_scalar(out=w_qls[:], in0=w_ql[:], scalar1=0.25,
                                    scalar2=None, op0=OP.mult)
            b_qls = const.tile([DL, 1], f32, tag="b_qls")
            nc.vector.tensor_scalar(out=b_qls[:], in0=b_ql[:], scalar1=0.25,
                                    scalar2=None, op0=OP.mult)

            # valid_lens -> clipped f16 row, broadcast across partitions
            vl_f32 = inp.tile([1, S], f32, tag="vl_f32")
            nc.vector.tensor_copy(vl_f32[:], vl_i[:])
            vl_f16 = inp.tile([1, S], f16, tag="vl_f16")
            nc.vector.tensor_scalar(out=vl_f16[:], in0=vl_f32[:], scalar1=float(S - 1),
                                    scalar2=None, op0=OP.min)
            nc.gpsimd.partition_broadcast(vl_bc[:], vl_f16[0:1, :])

            # global query index of each (partition, q-chunk): qbase + p + 128*qc
            qb_bc = inp.tile([128, 1], f32, tag="qb_bc")
            nc.gpsimd.partition_broadcast(qb_bc[:], qb_t[0:1, :])
            qi_i = inp.tile([128, QC], i32, tag="qi_i")
            nc.gpsimd.iota(qi_i[:], pattern=[[128, QC]], base=0, channel_multiplier=1)
            qi_f32 = inp.tile([128, QC], f32, tag="qi_f32")
            nc.vector.tensor_copy(qi_f32[:], qi_i[:])
            nc.vector.tensor_scalar(out=qidx[:], in0=qi_f32[:], scalar1=qb_bc[:, 0:1],
                                    scalar2=None, op0=OP.add)

            # transpose Q (full), K (full), Q_loc via PE
            for dst, stg, nch in ((qT, q_nat, KC), (qTl, ql_nat, QC), (kT, k_nat, KC)):
                for g in range(nch // 4):
                    ps = ps_tr.tile([128, 512], f32, tag="tr")
                    for j in range(4):
                        c = g * 4 + j
                        nc.tensor.transpose(ps[:, j * 128:(j + 1) * 128],
                                            stg[:, c * 128:(c + 1) * 128], ident[:])
                    nc.scalar.activation(dst[:, g * 512:(g + 1) * 512], ps[:], AF.Copy)

        # ---- projections ----
        qlowT = const.tile([DL, QH], f32, tag="qlowT")   # 0.25*(Wq_low^T Q_loc^T + bq)
        klowT = const.tile([DL, S], f32, tag="klowT")
        F_bc = const.tile([128, S], f32, tag="F_bc")

        with tc.tile_pool(name="proj", bufs=1) as proj, \
             tc.tile_pool(name="ps_p", bufs=2, space="PSUM") as ps_p, \
             tc.tile_pool(name="ps_f", bufs=2, space="PSUM") as ps_f:
            qpT = proj.tile([DL, S], f32, tag="qpT")
            F_row = proj.tile([1, S], f32, tag="F_row")     # exp(f[k])
            kpT = proj.tile([DL, S], f32, tag="kpT")
            prod = proj.tile([DL, S], f32, tag="prod")
            qT_r = proj.tile([128, S], f32r, tag="qT_r")
            kT_r = proj.tile([128, S], f32r, tag="kT_r")
            nc.vector.tensor_copy(qT_r[:], qT[:])
            nc.vector.tensor_copy(kT_r[:], kT[:])
            for s in range(2):
                ps = ps_p.tile([DL, 512], f32, tag="pp")
                nc.tensor.matmul(ps[:], R(w_qls[:]), R(qTl[:, s * 512:(s + 1) * 512]))
                nc.scalar.activation(qlowT[:, s * 512:(s + 1) * 512], ps[:], AF.Identity,
                                     bias=b_qls[:, 0:1])
            for s in range(4):
                sl = slice(s * 512, (s + 1) * 512)
                ps = ps_p.tile([DL, 512], f32, tag="pp")
                nc.tensor.matmul(ps[:], R(w_kl[:]), R(kT[:, sl]))
                nc.scalar.activation(klowT[:, sl], ps[:], AF.Identity, bias=b_kl[:, 0:1])
                ps2 = ps_p.tile([DL, 512], f32, tag="pp")
                nc.tensor.matmul(ps2[:], w_qh[:], qT_r[:, sl])
                nc.scalar.activation(qpT[:, sl], ps2[:], AF.Identity, bias=b_qh[:, 0:1])
                ps3 = ps_p.tile([DL, 512], f32, tag="pp")
                nc.tensor.matmul(ps3[:], w_kh[:], kT_r[:, sl])
                nc.scalar.activation(kpT[:, sl], ps3[:], AF.Identity, bias=b_kh[:, 0:1])
            nc.vector.tensor_tensor(out=prod[:], in0=qpT[:], in1=kpT[:], op=OP.mult)
            for s in range(4):
                sl = slice(s * 512, (s + 1) * 512)
                psf = ps_f.tile([1, 512], f32, tag="pf")
                nc.tensor.matmul(psf[:], R(ones16[:]), R(prod[:, sl]))
                nc.scalar.activation(F_row[0:1, sl], psf[:], AF.Exp, scale=0.25)
            nc.gpsimd.partition_broadcast(F_bc[:], F_row[0:1, :])

        # ---- phase A: scores -> exp -> top8 -> correct ;  phase B: transpose ----
        E_T = const.tile([128, KC * QH], f32r, tag="E_T")  # [k_in_chunk, kc*1024+q]
        cs = const.tile([128, 2 * KC], f32, tag="cs")     # per-(kc, wave) colsums

        colsum = const.tile([128, KC], f32, tag="colsum")
        ctot = const.tile([128, KC], f32, tag="ctot")
        rcol = const.tile([128, KC], f32, tag="rcol")
        v_sc = const.tile([128, KC * 128], f32r, tag="v_sc")
        KH = KC // 2

        def emit_colsum_half(hf):
            cs_in = cs_in0 if hf == 0 else cs_in1
            cs_out = cs_out0 if hf == 0 else cs_out1
            k0 = hf * KH
            csl = colsum[:, k0:k0 + KH]
            nc.vector.tensor_reduce(
                out=csl, in_=cs[:, 2 * k0:2 * (k0 + KH)].rearrange("p (k w) -> p k w", w=2),
                axis=mybir.AxisListType.X, op=OP.add)
            nc.sync.dma_start(cs_in[:, :], csl)
            if single_core:
                # profiling-only variant: stand in for the pairwise AllReduce
                nc.sync.dma_start(cs_out[:, :], cs_in[:, :])
            else:
                nc.gpsimd.collective_compute(
                    "AllReduce", OP.add, replica_groups=RG,
                    ins=[cs_in.ap()], outs=[cs_out.ap()],
                )
            nc.sync.dma_start(ctot[:, k0:k0 + KH], cs_out[:, :])
            nc.vector.reciprocal(rcol[:, k0:k0 + KH], ctot[:, k0:k0 + KH])
            for kc in range(k0, k0 + KH):
                nc.vector.tensor_scalar(out=v_sc[:, kc * 128:(kc + 1) * 128],
                                        in0=v_all[:, kc * 128:(kc + 1) * 128],
                                        scalar1=rcol[:, kc:kc + 1],
                                        scalar2=None, op0=OP.mult)

        ec = ctx.enter_context(tc.tile_pool(name="ec", bufs=6))
        ohp = ctx.enter_context(tc.tile_pool(name="oh", bufs=3))
        m8p = ctx.enter_context(tc.tile_pool(name="m8", bufs=2))
        t8p = ctx.enter_context(tc.tile_pool(name="t8", bufs=8))
        E_tiles = [None] * QC

        with tc.tile_pool(name="ps_sc", bufs=2, space="PSUM") as ps_sc:
            for qc in range(QC):
                oh = ohp.tile([128, S], f16, tag="oh")
                nc.vector.tensor_scalar(out=oh[:], in0=vl_bc[:],
                                        scalar1=qidx[:, qc:qc + 1],
                                        scalar2=None, op0=OP.is_equal)
                E = ec.tile([128, S], f32, tag="E")
                E_tiles[qc] = E
                for h2 in range(2):
                    ps = ps_sc.tile([128, 1024], f32, tag="sc")
                    for s in range(2):
                        col = h2 * 1024 + s * 512
                        nc.tensor.matmul(ps[:, s * 512:(s + 1) * 512],
                                         R(qlowT[:, qc * 128:(qc + 1) * 128]),
                                         R(klowT[:, col:col + 512]),
                                         start=True, stop=False)
                        nc.tensor.matmul(ps[:, s * 512:(s + 1) * 512],
                                         negI[:], oh[:, col:col + 512],
                                         start=False, stop=True)
                    nc.scalar.activation(E[:, h2 * 1024:(h2 + 1) * 1024], ps[:], AF.Exp)
                t8 = t8p.tile([128, 8], f32, tag="t8")
                nc.vector.max(out=t8[:], in_=E[:])
                m8 = m8p.tile([128, S], mybir.dt.uint8, tag="m8")
                nc.gpsimd.tensor_scalar(out=m8[:], in0=E[:], scalar1=t8[:, 7:8],
                                        scalar2=None, op0=OP.is_ge)
                nc.vector.copy_predicated(out=E[:], mask=m8[:], data=F_bc[:])

                if qc == 3 or qc == 7:
                    w = qc // 4
                    for kc in range(KC):
                        ps = ps_tr.tile([128, 512], f32, tag="tr")
                        for j in range(4):
                            Ej = E_tiles[w * 4 + j]
                            nc.tensor.transpose(ps[:, j * 128:(j + 1) * 128],
                                                Ej[:, kc * 128:(kc + 1) * 128],
                                                ident[:])
                        idx = kc * 2 + w
                        dst = E_T[:, kc * QH + w * 512: kc * QH + w * 512 + 512]
                        if kc % 2 == 0:
                            nc.scalar.activation(dst, ps[:], AF.Copy,
                                                 accum_out=cs[:, idx:idx + 1])
                        else:
                            nc.vector.tensor_scalar(out=dst, in0=ps[:], scalar1=0.0,
                                                    scalar2=None, op0=OP.add, op1=OP.add,
                                                    accum_out=cs[:, idx:idx + 1])
                        if qc == 7 and kc == KC // 2 - 1:
                            emit_colsum_half(0)
                    if qc == 7:
                        emit_colsum_half(1)

        # ---- phase D: out^T = sum_k v_sc[k,:]^T E_T[k,:] ; transpose back; store ----
        outT = const.tile([128, QH], f32, tag="outT")   # [d, q_local]
        out_sb = const.tile([128, QH], f32, tag="out_sb")  # [q_in_chunk, qc*128+d]
        with tc.tile_pool(name="ps_o", bufs=2, space="PSUM") as ps_o:
            for h2 in range(2):
                po = ps_o.tile([128, 512], f32, tag="po")
                for kc in range(KC):
                    nc.tensor.matmul(po[:], R(v_sc[:, kc * 128:(kc + 1) * 128]),
                                     R(E_T[:, kc * QH + h2 * 512: kc * QH + h2 * 512 + 512]),
                                     start=(kc == 0), stop=(kc == KC - 1))
                nc.scalar.activation(outT[:, h2 * 512:(h2 + 1) * 512], po[:], AF.Copy)
            for g in range(2):
                ps = ps_tr.tile([128, 512], f32, tag="tr")
                for j in range(4):
                    qc = g * 4 + j
                    nc.tensor.transpose(ps[:, j * 128:(j + 1) * 128],
                                        outT[:, qc * 128:(qc + 1) * 128], ident[:])
                nc.scalar.activation(out_sb[:, g * 512:(g + 1) * 512], ps[:], AF.Copy)
        # one rearranged store per half, on separate DGE rings
        nc.sync.dma_start(
            t_out[0:QH // 2, :].rearrange("(c p) d -> p c d", p=128),
            out_sb[:, 0:QH // 2].rearrange("p (c d) -> p c d", d=128))
        nc.scalar.dma_start(
            t_out[QH // 2:QH, :].rearrange("(c p) d -> p c d", p=128),
            out_sb[:, QH // 2:QH].rearrange("p (c d) -> p c d", d=128))

    nc.compile()
    return nc


def _make_in_maps(inputs):
    q = np.ascontiguousarray(np.asarray(inputs["queries"], dtype=np.float32))
    k = np.ascontiguousarray(np.asarray(inputs["keys"], dtype=np.float32))
    v = np.ascontiguousarray(np.asarray(inputs["values"], dtype=np.float32))
    vl = np.ascontiguousarray(np.asarray(inputs["valid_lens"], dtype=np.int32))
    ws = {n: np.ascontiguousarray(np.asarray(inputs[n], dtype=np.float32))
          for n in ("Wq_low", "Wk_low", "Wq_high", "Wk_high")}
    bs = {n: np.ascontiguousarray(
            np.asarray(inputs[n], dtype=np.float32).reshape(DL, 1))
          for n in ("bq_low", "bk_low", "bq_high", "bk_high")}
    in_maps = []
    for c in range(N_CORES):
        b, h = c // 2, c % 2
        m = {
            "queries": q[b],
            "queries_loc": np.ascontiguousarray(q[b, h * QH:(h + 1) * QH]),
            "keys": k[b],
            "values": v[b],
            "valid_lens": vl[b].reshape(1, S),
            "qbase": np.array([[float(h * QH)]], np.float32),
        }
        m.update(ws)
        m.update(bs)
        in_maps.append(m)
    return in_maps


def _run(inputs) -> np.ndarray:
    from concourse import bass_utils

    if "nc" not in _cache:
        _cache["nc"] = _build()
    nc = _cache["nc"]
    in_maps = _make_in_maps(inputs)
    res = bass_utils.run_bass_kernel_spmd(nc, in_maps, core_ids=list(range(N_CORES)))
    out = np.empty((B, S, D), np.float32)
    for c in range(N_CORES):
        b, h = c // 2, c % 2
        out[b, h * QH:(h + 1) * QH, :] = res.results[c]["out"]
    return out


def kernel(**inputs) -> np.ndarray:
    """Entry point. The axon/NRT stack occasionally starts a process with the
    device in an unrecoverable state (collective desync left by a previous
    process); every execution in such a process fails. A fresh process heals
    it, so retry in subprocesses on failure."""
    try:
        return _run(inputs)
    except Exception as e:  # noqa: BLE001 - device-level flake, retry fresh
        import subprocess
        import tempfile

        last = e
        for _ in range(3):
            with tempfile.TemporaryDirectory() as td:
                inp = os.path.join(td, "in.npz")
                outp = os.path.join(td, "out.npz")
                np.savez(inp, **{k: np.asarray(v) for k, v in inputs.items()})
                r = subprocess.run(
                    [sys.executable, os.path.abspath(__file__), "--subproc", inp, outp],
                    capture_output=True, timeout=900)
                if r.returncode == 0 and os.path.exists(outp):
                    return np.load(outp)["out"]
                last = RuntimeError(
                    f"subprocess retry failed: {r.stderr[-2000:]!r}")
        raise last


if __name__ == "__main__" and len(sys.argv) >= 4 and sys.argv[1] == "--subproc":
    _data = np.load(sys.argv[2])
    _out = _run({k: _data[k] for k in _data.files})
    np.savez(sys.argv[3], out=_out)
